# revision 51
# baseline (speedup 1.0000x reference)
"""Dilated self-attention Trainium2 kernel.

Decomposition (validated exactly against the reference in numpy):
  - x: [4, 8192, 64]. Per batch element there are 7 causal attention windows of
    m=2048 tokens: 4 contiguous (stride 1), 2 stride-2, 1 stride-4.
  - 8 cores, 2 per batch element. Core (2b+h) owns output positions
    [h*4096, (h+1)*4096) of batch b and computes the 4 window-tasks whose
    outputs land entirely in that range:
      W1 = A window  xa[0:2048]          (stride-1 tokens, full 2048 queries)
      W2 = A window  xa[2048:4096]
      W3 = B window  xa[0::2]            (stride-2 tokens, full queries)
      W4 = C window upper-half queries [1024,2048) over xc tokens
    For even cores (h=0) the host zero-pads the first 1024 C tokens, which
    makes W4 structurally identical on all cores: zero tokens have K=0
    (exp(0)=1 harmless) and a zero entry in the ones-column of V-hat, so they
    contribute exactly nothing to numerator and denominator.
  - Softmax without max-subtraction (logits bounded ~±15, exp safe in fp32).
    P @ [V | 1] yields the un-normalized output o*d and the denominator d in
    one matmul. Per-window results accumulate position-major into a DRAM
    scratch via accumulate-DMA; a final reciprocal-multiply mixes them.
  - Matmul operands are bitcast to float32r (full PE rate at free-dim >= 256,
    near-fp32 precision).
"""

import sys

if "/opt/trn_rl_repo" not in sys.path:
    sys.path.insert(0, "/opt/trn_rl_repo")

import numpy as np

P = 128
C = 64
C1 = 65          # c + ones column
C2 = 66          # padded even width for the fp32r V-hat/output chain
N = 8192
B = 4
M = 2048         # window length
HALF = 4096
NCORES = 8

_CACHE = {}


def _build():
    import concourse.bass as bass
    import concourse.mybir as mybir
    from concourse.bacc import Bacc
    from concourse.tile import TileContext
    from concourse.masks import make_identity

    fp32 = mybir.dt.float32
    r = mybir.dt.float32r
    Exp = mybir.ActivationFunctionType.Exp
    add_op = mybir.AluOpType.add
    mult = mybir.AluOpType.mult
    is_ge = mybir.AluOpType.is_ge

    nc = Bacc()
    xa_d = nc.dram_tensor("xa", [HALF, C1], r, kind="ExternalInput")
    xc_d = nc.dram_tensor("xc", [M, C1], r, kind="ExternalInput")
    wq_d = nc.dram_tensor("wq", [C, C], r, kind="ExternalInput")
    wk_d = nc.dram_tensor("wk", [C, C], r, kind="ExternalInput")
    wv_d = nc.dram_tensor("wv", [C1, C2], r, kind="ExternalInput")
    out_d = nc.dram_tensor("out", [HALF, C], fp32, kind="ExternalOutput")

    with TileContext(nc) as tc:
        with (
            tc.tile_pool(name="wpool", bufs=1) as wpool,
            tc.tile_pool(name="xpool", bufs=1) as xpool,
            tc.tile_pool(name="tpool", bufs=1) as tpool,
            tc.tile_pool(name="kq", bufs=2) as kq,
            tc.tile_pool(name="vpool", bufs=3) as vpool,
            tc.tile_pool(name="ptpool", bufs=3) as ptpool,
            tc.tile_pool(name="opool", bufs=4) as opool,
            tc.tile_pool(name="odpool", bufs=4) as odpool,
            tc.tile_pool(name="mix", bufs=2) as mix,
            tc.tile_pool(name="spool", bufs=3, space="PSUM") as spool,
            tc.tile_pool(name="mpool", bufs=2, space="PSUM") as mpool,
            tc.tile_pool(name="dpool", bufs=1, space="DRAM") as dpool,
        ):
            acc_q = [
                dpool.tile([1024, C2], fp32, name=f"accq{ch}")
                for ch in range(4)
            ]

            ident32 = wpool.tile([P, P], fp32, tag="id32")
            make_identity(nc, ident32)
            ident = wpool.tile([P, P], r, tag="id")
            nc.vector.tensor_copy(ident[:], ident32[:])
            wq_sb = wpool.tile([C, C], r, tag="wq")
            wk_sb = wpool.tile([C, C], r, tag="wk")
            wv_sb = wpool.tile([C1, C2], r, tag="wv")
            # ---- load + transpose token matrices (xa first: critical path) ----
            xa_nat = xpool.tile([P, HALF // P, C1], r, tag="xa")
            xa_r = xa_d[:].rearrange("(t p) c -> p t c", p=P)
            for lo, hi in ((0, 4), (4, 8), (8, 16), (16, 32)):
                nc.sync.dma_start(
                    xa_nat[:, lo:hi, :], xa_r[:, lo:hi, :]
                )
            nc.gpsimd.dma_start(wq_sb[:], wq_d[:])
            nc.gpsimd.dma_start(wk_sb[:], wk_d[:])
            nc.gpsimd.dma_start(wv_sb[:], wv_d[:])
            xc_nat = xpool.tile([P, M // P, C1], r, tag="xc")
            xc_r = xc_d[:].rearrange("(t p) c -> p t c", p=P)
            for ch in range(2):
                nc.gpsimd.dma_start(
                    xc_nat[:, ch * 8:(ch + 1) * 8, :], xc_r[:, ch * 8:(ch + 1) * 8, :]
                )

            XAT = tpool.tile([C1, HALF], r, tag="xat")
            for t8 in range(HALF // 1024):
                ps = spool.tile([P, 1024], r, tag="s")
                for i in range(8):
                    t = t8 * 8 + i
                    nc.tensor.transpose(
                        ps[:C1, i * P:(i + 1) * P],
                        xa_nat[:, t, :],
                        ident,
                    )
                nc.scalar.copy(XAT[:, t8 * 1024:(t8 + 1) * 1024], ps[:C1])

            XCT = tpool.tile([C1, M], r, tag="xct")

            # strided token views (window-local index j -> column)
            tok_W3 = XAT.rearrange("c (j s) -> c j s", s=2)[:, :, 0]  # [C1, 2048]

            def emit_qkv(tokT, q0):
                KT = kq.tile([C, M], r, tag="kt")
                for s4 in range(M // 512):
                    ps = mpool.tile([P, 512], fp32, tag="m")
                    nc.tensor.matmul(
                        ps[:C],
                        wk_sb,
                        tokT[:C, s4 * 512:(s4 + 1) * 512],
                        start=True, stop=True,
                    )
                    nc.vector.tensor_copy(KT[:, s4 * 512:(s4 + 1) * 512], ps[:C])
                nq = M - q0
                QT = kq.tile([C, M], r, tag="qt")
                for s4 in range(nq // 512):
                    ps = mpool.tile([P, 512], fp32, tag="m")
                    nc.tensor.matmul(
                        ps[:C],
                        wq_sb,
                        tokT[:C, q0 + s4 * 512:q0 + (s4 + 1) * 512],
                        start=True, stop=True,
                    )
                    nc.vector.tensor_copy(QT[:, s4 * 512:(s4 + 1) * 512], ps[:C])
                VH = vpool.tile([P, M // P, C2], r, tag="vh")
                for t4 in range(4):
                    ps = mpool.tile([P, 512], fp32, tag="m")
                    for i in range(4):
                        t = t4 * 4 + i
                        nc.tensor.matmul(
                            ps[:P, i * C2:(i + 1) * C2],
                            tokT[:, t * P:(t + 1) * P],
                            wv_sb,
                            start=True, stop=True,
                        )
                    nc.vector.tensor_copy(
                        VH[:, t4 * 4:(t4 + 1) * 4, :],
                        ps[:, :4 * C2].rearrange("p (a c) -> p a c", c=C2),
                    )
                return KT, QT, VH

            def emit_attn(KT, QT, VH, q0, st, base):
                for qc in range(q0 // 512, M // 512):
                    nk = 4 * (qc + 1)
                    qrel = qc * 512 - q0
                    PT = ptpool.tile([P, 16, 512], r, tag="pt")
                    kh_order = [nk // 2 - 2, nk // 2 - 1] + list(range(nk // 2 - 2))
                    for kh in kh_order:
                        sp = spool.tile([P, 1024], fp32, tag="s")
                        for i in range(2):
                            kc = 2 * kh + i
                            nc.tensor.matmul(
                                sp[:, i * 512:(i + 1) * 512],
                                KT[:, kc * P:(kc + 1) * P],
                                QT[:, qrel:qrel + 512],
                                start=True, stop=True,
                            )
                        nc.scalar.activation(
                            PT[:, 2 * kh:2 * kh + 2, :], sp, Exp, scale=0.125
                        )
                        if kh >= nk // 2 - 2:
                            for i in range(2):
                                kc = 2 * kh + i
                                d = kc - (nk - 4)
                                nc.gpsimd.affine_select(
                                    out=PT[:, kc, :],
                                    in_=PT[:, kc, :],
                                    pattern=[[1, 512]],
                                    compare_op=is_ge,
                                    fill=0.0,
                                    base=-P * d,
                                    channel_multiplier=-1,
                                )
                    pv = mpool.tile([P, 512], fp32, tag="m")
                    kc_order = list(range(nk - 4, nk)) + list(range(nk - 4))
                    for j, kc in enumerate(kc_order):
                        nc.tensor.matmul(
                            pv[:C2],
                            VH[:, kc, :],
                            PT[:, kc, :],
                            start=(j == 0), stop=(j == nk - 1),
                        )
                    osb = opool.tile([C2, 512], r, tag="osb")
                    nc.any.tensor_copy(out=osb, in_=pv[:C2])
                    tp = mpool.tile([P, 512], r, tag="m")
                    for i in range(4):
                        nc.tensor.transpose(
                            tp[:P, i * C2:(i + 1) * C2],
                            osb[:, i * P:(i + 1) * P],
                            ident[:C2, :C2],
                        )
                    od = odpool.tile([P, 4, C2], fp32, tag="od")
                    nc.any.tensor_copy(
                        out=od, in_=tp[:, :4 * C2].rearrange("p (a c) -> p a c", c=C2)
                    )
                    if st == 1:
                        r0 = base + qrel
                        dest = acc_q[r0 // 1024][r0 % 1024:r0 % 1024 + 512]
                        dest = dest.rearrange("(t p) c -> p t c", p=P)
                        nc.sync.dma_start(dest, od)
                    elif st == 2:
                        ch = (2 * qrel) // 1024
                        dest = acc_q[ch].rearrange(
                            "(j s) c -> j s c", s=2
                        )[:, 0, :].rearrange("(t p) c -> p t c", p=P)
                        nc.gpsimd.dma_start(dest, od, accum_op=add_op)
                    else:  # st == 4: one od spans two 1024-row chunks
                        for hh in range(2):
                            ch = (4 * qrel) // 1024 + hh
                            dest = acc_q[ch].rearrange(
                                "(j s) c -> j s c", s=4
                            )[:, 0, :].rearrange("(t p) c -> p t c", p=P)
                            nc.gpsimd.dma_start(
                                dest, od[:, 2 * hh:2 * hh + 2, :],
                                accum_op=add_op,
                            )

            wins = [
                (XAT[:, 0:M], 0, 1, 0),
                (XAT[:, M:2 * M], 0, 1, M),
                (tok_W3, 0, 2, 0),
                (XCT, 1024, 4, 0),
            ]
            def emit_xct():
                for t8 in range(M // 1024):
                    ps = spool.tile([P, 1024], r, tag="s")
                    for i in range(8):
                        t = t8 * 8 + i
                        nc.tensor.transpose(
                            ps[:C1, i * P:(i + 1) * P],
                            xc_nat[:, t, :],
                            ident,
                        )
                    nc.vector.tensor_copy(
                        XCT[:, t8 * 1024:(t8 + 1) * 1024], ps[:C1]
                    )

            pend = None
            for wi, (tokT, q0, st, base) in enumerate(wins):
                if wi == 2:
                    emit_xct()
                h = emit_qkv(tokT, q0)
                if pend is not None:
                    (pKT, pQT, pVH), (pq0, pst, pbase) = pend
                    emit_attn(pKT, pQT, pVH, pq0, pst, pbase)
                pend = (h, (q0, st, base))
            (pKT, pQT, pVH), (pq0, pst, pbase) = pend
            emit_attn(pKT, pQT, pVH, pq0, pst, pbase)

            # ---- final mix: out = acc[:, :64] / acc[:, 64], in 4 chunks ----
            T = HALF // P  # 32
            for ch in range(4):
                rows = slice(ch * 1024, (ch + 1) * 1024)
                accsb = mix.tile([P, T // 4, C2], fp32, tag="accsb")
                nc.sync.dma_start(
                    accsb[:],
                    acc_q[ch][:].rearrange("(t p) c -> p t c", p=P),
                )
                rec = mix.tile([P, T // 4], fp32, tag="rec")
                nc.vector.reciprocal(rec, accsb[:, :, C])
                outsb = mix.tile([P, T // 4, C], fp32, tag="outsb")
                nc.vector.tensor_tensor(
                    outsb,
                    accsb[:, :, 0:C],
                    rec[:, :, None].to_broadcast((P, T // 4, C)),
                    mult,
                )
                nc.sync.dma_start(
                    out_d[rows].rearrange("(t p) c -> p t c", p=P), outsb[:]
                )

    nc.compile()
    return nc


def _host_inputs(x, Wq, Wk, Wv):
    """Build the 8 per-core input maps."""
    Wv_h = np.zeros((C1, C2), np.float32)
    Wv_h[:C, :C] = Wv
    Wv_h[C, C] = 1.0
    in_maps = []
    for core in range(NCORES):
        b, h = core // 2, core % 2
        xa = np.ones((HALF, C1), np.float32)
        xa[:, :C] = x[b, h * HALF:(h + 1) * HALF]
        xc = np.zeros((M, C1), np.float32)
        if h == 1:
            xc[:, :C] = x[b, 0::4]
            xc[:, C] = 1.0
        else:
            xc[1024:, :C] = x[b, 0:HALF:4]
            xc[1024:, C] = 1.0
        in_maps.append({
            "xa": np.ascontiguousarray(xa),
            "xc": np.ascontiguousarray(xc),
            "wq": np.ascontiguousarray(Wq.astype(np.float32)),
            "wk": np.ascontiguousarray(Wk.astype(np.float32)),
            "wv": Wv_h,
        })
    return in_maps


def _run(x, Wq, Wk, Wv, trace=False):
    from concourse.bass_utils import run_bass_kernel_spmd

    if "nc" not in _CACHE:
        _CACHE["nc"] = _build()
    nc = _CACHE["nc"]
    in_maps = _host_inputs(x, Wq, Wk, Wv)
    res = run_bass_kernel_spmd(
        nc, in_maps, core_ids=list(range(NCORES)), trace=trace
    )
    out = np.zeros((B, N, C), np.float32)
    for core in range(NCORES):
        b, h = core // 2, core % 2
        out[b, h * HALF:(h + 1) * HALF] = res.results[core]["out"]
    return out, res


def kernel(x, Wq, Wk, Wv):
    x = np.asarray(x, dtype=np.float32)
    out, _ = _run(x, np.asarray(Wq), np.asarray(Wk), np.asarray(Wv))
    return out
